# revision 13
# baseline (speedup 1.0000x reference)
"""DGCNN forward on 8 Trainium2 NeuronCores (Bass/Tile), pure data parallel.

Each core processes one sample (N=1024 points, K=20 neighbors).

Algorithmic mapping per EdgeConv layer (weights BN-folded on host):
  y[:,n,j] = Wd@(x_nbr - x_ctr) + Wc@x_ctr   (1x1 conv on edge features)
           = Wd@x[:,idx[n,j]] + (Wc-Wd)@x[:,n]
  After folding the (eval-mode) BN scale s and bias into the weights, and
  because max over neighbors commutes with the monotone LeakyReLU:
  out[:,n] = lrelu( max_j u[:,idx[n,j]] + v[:,n] )
  with u = (s*Wd)@x + (s*bias + b)  and  v = (s*(Wc-Wd))@x.

  KNN row scores: top-20 of  s[n,m] = <x_n,x_m> - ||x_m||^2/2  (equivalent
  ordering to the reference's -||x_n-x_m||^2 per row).

Top-20 per row: 3 rounds of DVE max8 / max_index / match_replace.
Neighbor gather: gpsimd dma_gather of u^T rows from DRAM, split over the
4 SWDGE queues; reduce-max over the 20 gathered rows on the Pool engine.
"""

import os

import numpy as np

N = 1024
K = 20
NCORES = 8
EPS = 1e-5
SLOPE = 0.01
NEG = -3.0e38

# (C_in, O) per edge conv layer
EDGE_LAYERS = [(3, 64), (64, 64), (64, 128), (128, 256)]

_CACHE = {}
LAST_RESULTS = None


def _build():
    import concourse.bass as bass
    import concourse.mybir as mybir
    import concourse.tile as tile
    from concourse import bacc

    dt = mybir.dt
    f32 = dt.float32
    u16 = dt.uint16
    f16 = dt.float16
    i16 = dt.int16
    Alu = mybir.AluOpType
    Act = mybir.ActivationFunctionType
    AX = mybir.AxisListType

    nc = bacc.Bacc("TRN2", target_bir_lowering=False, debug=False,
                   num_swdge_queues=4)

    # ---------------- DRAM I/O ----------------
    xin = nc.dram_tensor("xin", [N, 3], f32, kind="ExternalInput")
    ATs, BTs, c0s = [], [], []
    for li, (C, O) in enumerate(EDGE_LAYERS):
        ATs.append(nc.dram_tensor(f"AT{li}", [C, O], f32, kind="ExternalInput"))
        BTs.append(nc.dram_tensor(f"BT{li}", [C, O], f32, kind="ExternalInput"))
        c0s.append(nc.dram_tensor(f"c0{li}", [1, O], f32, kind="ExternalInput"))
    w5T = nc.dram_tensor("w5T", [512, 1024], f32, kind="ExternalInput")
    l1T = nc.dram_tensor("l1T", [1024, 512], f32, kind="ExternalInput")
    b6 = nc.dram_tensor("b6", [1, 512], f32, kind="ExternalInput")
    l2T = nc.dram_tensor("l2T", [512, 256], f32, kind="ExternalInput")
    c7 = nc.dram_tensor("c7", [1, 256], f32, kind="ExternalInput")
    l3T = nc.dram_tensor("l3T", [256, 40], f32, kind="ExternalInput")
    b8 = nc.dram_tensor("b8", [1, 40], f32, kind="ExternalInput")
    out_d = nc.dram_tensor("out", [40, 1], f32, kind="ExternalOutput")

    with tile.TileContext(nc) as tc, __import__("contextlib").ExitStack() as ctx:
        const = ctx.enter_context(tc.tile_pool(name="const", bufs=1))
        xpool = ctx.enter_context(tc.tile_pool(name="xpool", bufs=1))
        work = ctx.enter_context(tc.tile_pool(name="work", bufs=3))
        srow_p = ctx.enter_context(tc.tile_pool(name="srow", bufs=3))
        gth_p = ctx.enter_context(tc.tile_pool(name="gth", bufs=2))
        vt_p = ctx.enter_context(tc.tile_pool(name="vt", bufs=3))
        small = ctx.enter_context(tc.tile_pool(name="small", bufs=4))
        mm = ctx.enter_context(tc.tile_pool(name="mm", bufs=4, space="PSUM"))
        sm = ctx.enter_context(tc.tile_pool(name="sm", bufs=3, space="PSUM"))
        dram = ctx.enter_context(tc.tile_pool(name="dram", bufs=2, space="DRAM"))
        dram_s = ctx.enter_context(tc.tile_pool(name="dram_s", bufs=3, space="DRAM"))

        # ------------- constants into SBUF -------------
        def load_const(name, dram_t, shape=None):
            t = const.tile(list(shape or dram_t.shape), f32, tag=name)
            nc.sync.dma_start(t[:], dram_t.ap())
            return t

        AT_sb = [load_const(f"AT{i}", ATs[i]) for i in range(4)]
        BT_sb = [load_const(f"BT{i}", BTs[i]) for i in range(4)]
        c0_sb = [load_const(f"c0{i}", c0s[i]) for i in range(4)]
        b6_sb = load_const("b6", b6)
        c7_sb = load_const("c7", c7)
        b8_sb = load_const("b8", b8)

        # w5T: 5 K-chunks matching [x1(64), x2(64), x3(128), x4a(128), x4b(128)]
        w5_rows = [(0, 64), (64, 128), (128, 256), (256, 384), (384, 512)]
        w5_sb = []
        for i, (r0, r1) in enumerate(w5_rows):
            t = const.tile([r1 - r0, 1024], f32, tag=f"w5_{i}")
            nc.sync.dma_start(t[:], w5T.ap()[r0:r1, :])
            w5_sb.append(t)
        l1_sb = []
        for k in range(8):
            t = const.tile([128, 512], f32, tag=f"l1_{k}")
            nc.sync.dma_start(t[:], l1T.ap()[k * 128:(k + 1) * 128, :])
            l1_sb.append(t)
        l2_sb = []
        for k in range(4):
            t = const.tile([128, 256], f32, tag=f"l2_{k}")
            nc.sync.dma_start(t[:], l2T.ap()[k * 128:(k + 1) * 128, :])
            l2_sb.append(t)
        l3_sb = []
        for k in range(2):
            t = const.tile([128, 40], f32, tag=f"l3_{k}")
            nc.sync.dma_start(t[:], l3T.ap()[k * 128:(k + 1) * 128, :])
            l3_sb.append(t)

        ones_col = const.tile([128, 1], f32, tag="ones_col")
        nc.vector.memset(ones_col[:], 1.0)
        ones_row = const.tile([1, 128], f32, tag="ones_row")
        nc.vector.memset(ones_row[:], 1.0)

        # persistent channel-major feature tiles
        x0 = xpool.tile([3, 1024], f32, tag="x0")
        x1 = xpool.tile([64, 1024], f32, tag="x1")
        x2 = xpool.tile([64, 1024], f32, tag="x2")
        x3 = xpool.tile([128, 1024], f32, tag="x3")
        x4a = xpool.tile([128, 1024], f32, tag="x4a")
        x4b = xpool.tile([128, 1024], f32, tag="x4b")
        gp = xpool.tile([128, 8], f32, tag="gp")

        # load x (point-major in DRAM) transposed into channel-major [3, 1024]
        nc.sync.dma_start(x0[:], xin.ap().rearrange("n c -> c n"))

        # ---------------- edge conv layer ----------------
        def edge_layer(li, xch, C, O, xouts):
            """xch: [C, 1024] channel-major SBUF AP.
            xouts: list of [rows, 1024] channel-major target tiles, one per
            128-channel block (block f holds channels f*128..)."""
            Opad = max(O, 128)       # gather element = Opad fp16 (>=256B)
            Of = Opad // 128

            # u^T rows (fp16) to DRAM as the gather source
            uTd = dram.tile([N, 256], f16, tag="uTd")
            u_src = uTd[:, 0:Opad]   # ap[0] step 256 -> elem_step

            # xx = colsum(x*x); nxx = -xx/2 as a [1, 1024] row
            xsq = work.tile([C, 1024], f32, tag="xsq")
            nc.vector.tensor_mul(xsq[:], xch, xch)
            nxx = work.tile([1, 1024], f32, tag="nxx")
            for h in range(2):
                ps = mm.tile([1, 512], f32, tag="mm")
                nc.tensor.matmul(ps[:], ones_col[0:C, :], xsq[:, h * 512:(h + 1) * 512])
                nc.scalar.activation(nxx[0:1, h * 512:(h + 1) * 512], ps[:],
                                     Act.Copy, scale=-0.5)

            # u^T per 128-point chunk (point-major, fp16, with c0 folded in)
            for m in range(8):
                csl = slice(m * 128, (m + 1) * 128)
                pu = sm.tile([128, O], f32, tag="sm")
                nc.tensor.matmul(pu[:], xch[:, csl], AT_sb[li][:], start=True, stop=False)
                nc.tensor.matmul(pu[:], ones_row[:], c0_sb[li][:], start=False, stop=True)
                uT = work.tile([128, O], f16, tag="uT")
                nc.scalar.activation(uT[:], pu[:], Act.Copy)
                nc.sync.dma_start(uTd[csl, 0:O], uT[:])

            # v channel-major [O, 1024] (f32)
            vs = []
            for f in range(Of if O >= 128 else 1):
                osl = slice(f * 128, min((f + 1) * 128, O))
                orows = osl.stop - osl.start
                vt = vt_p.tile([128, 1024], f32, tag="vt")
                for h in range(2):
                    nsl = slice(h * 512, (h + 1) * 512)
                    pv = mm.tile([128, 512], f32, tag="mm")
                    nc.tensor.matmul(pv[0:orows, :], BT_sb[li][:, osl], xch[:, nsl])
                    nc.scalar.activation(vt[0:orows, nsl], pv[0:orows, :], Act.Copy)
                vs.append(vt)

            # per-chunk: dist row block -> top20 -> gather -> max -> lrelu
            for m in range(8):
                csl = slice(m * 128, (m + 1) * 128)
                srow = srow_p.tile([128, 1024], f32, tag="srow")
                for h in range(2):
                    nsl = slice(h * 512, (h + 1) * 512)
                    pd = mm.tile([128, 512], f32, tag="mm")
                    nc.tensor.matmul(pd[:], xch[:, csl], xch[:, nsl],
                                     start=True, stop=False)
                    nc.tensor.matmul(pd[:], ones_row[:], nxx[0:1, nsl],
                                     start=False, stop=True)
                    nc.scalar.activation(srow[:, nsl], pd[:], Act.Copy)

                idx24 = small.tile([128, 24], u16, tag="idx24")
                for r in range(3):
                    m8 = small.tile([128, 8], f32, tag="m8")
                    nc.vector.max(m8[:], srow[:])
                    nc.vector.max_index(idx24[:, r * 8:r * 8 + 8], m8[:], srow[:])
                    if r < 2:
                        nc.vector.match_replace(srow[:], m8[:], srow[:], NEG)

                # top-20 = rounds 0,1 and first 4 of round 2 -> idx24[:, :20].
                # DRAM round-trip to build dma_gather's wrapped int16 layout:
                #   flat[i] = idx[point i//20][nbr i%20]  (n-major),
                #   sbuf[q, s] = flat[s*16+q], replicated to all 128 partitions.
                F2 = dram_s.tile([2560], u16, tag="F2")
                nc.sync.dma_start(F2[:].rearrange("(p c) -> p c", p=128), idx24[:, 0:20])
                idxs = small.tile([128, 160], i16, tag="idxs")
                rd = F2[:].bitcast(i16).rearrange("(s q) -> q s", q=16)
                for g in range(8):
                    nc.sync.dma_start(idxs[g * 16:(g + 1) * 16, :], rd)

                # gather u^T rows channel-major (transpose mode), split over
                # the 4 SWDGE queues (queue q: points 32q..32q+32 of chunk)
                mkT = small.tile([128, Of * 128], f16, tag="mkT")
                mkT3 = mkT[:].rearrange("p (f n) -> p f n", f=Of)
                for q in range(4):
                    gq = gth_p.tile([128, Of * 640], f16, tag="gth")
                    nc.gpsimd.dma_gather(
                        gq[:].rearrange("p (f i) -> p f i", f=Of),
                        u_src,
                        idxs[:, q * 40:(q + 1) * 40],
                        640, 640, Opad, elem_step=256, transpose=True,
                        queue_num=q,
                    )
                    nc.vector.reduce_max(
                        mkT3[:, :, q * 32:(q + 1) * 32],
                        gq[:].rearrange("p (f n j) -> p f n j", f=Of, j=20),
                        axis=AX.X)

                # x_next = lrelu(mk + v), written channel-major per 128-block
                for f, (xt, rows) in enumerate(xouts):
                    z = small.tile([128, 128], f32, tag="z")
                    nc.vector.tensor_add(z[0:rows, :], mkT[0:rows, f * 128:f * 128 + 128],
                                         vs[f][0:rows, csl])
                    nc.vector.scalar_tensor_tensor(
                        out=xt[0:rows, csl], in0=z[0:rows, :], scalar=SLOPE,
                        in1=z[0:rows, :], op0=Alu.mult, op1=Alu.max)

        edge_layer(0, x0[:], 3, 64, [(x1, 64)])
        edge_layer(1, x1[:], 64, 64, [(x2, 64)])
        edge_layer(2, x2[:], 64, 128, [(x3, 128)])
        edge_layer(3, x3[:], 128, 256, [(x4a, 128), (x4b, 128)])

        # ---------------- conv5 (512 -> 1024) + global max pool ----------------
        xc_chunks = [x1, x2, x3, x4a, x4b]
        for mo in range(8):
            msl = slice(mo * 128, (mo + 1) * 128)
            gp2 = small.tile([128, 2], f32, tag="gp2")
            for h in range(2):
                nsl = slice(h * 512, (h + 1) * 512)
                pe = mm.tile([128, 512], f32, tag="mm")
                for k in range(5):
                    nc.tensor.matmul(pe[:], w5_sb[k][:, msl], xc_chunks[k][:, nsl],
                                     start=(k == 0), stop=(k == 4))
                nc.vector.reduce_max(gp2[:, h:h + 1], pe[:], axis=AX.X)
            nc.vector.tensor_tensor(out=gp[:, mo:mo + 1], in0=gp2[:, 0:1],
                                    in1=gp2[:, 1:2], op=Alu.max)

        # ---------------- MLP head ----------------
        y1 = small.tile([128, 4], f32, tag="y1")
        for mt in range(4):
            msl = slice(mt * 128, (mt + 1) * 128)
            p1 = sm.tile([128, 1], f32, tag="sm")
            for k in range(8):
                nc.tensor.matmul(p1[:], l1_sb[k][:, msl], gp[:, k:k + 1],
                                 start=(k == 0), stop=False)
            nc.tensor.matmul(p1[:], b6_sb[0:1, msl], ones_row[0:1, 0:1],
                             start=False, stop=True)
            t1 = small.tile([128, 1], f32, tag="t1")
            nc.scalar.activation(t1[:], p1[:], Act.Copy)
            nc.vector.scalar_tensor_tensor(
                out=y1[:, mt:mt + 1], in0=t1[:], scalar=SLOPE, in1=t1[:],
                op0=Alu.mult, op1=Alu.max)

        y2 = small.tile([128, 2], f32, tag="y2")
        for mt in range(2):
            msl = slice(mt * 128, (mt + 1) * 128)
            p2 = sm.tile([128, 1], f32, tag="sm")
            for k in range(4):
                nc.tensor.matmul(p2[:], l2_sb[k][:, msl], y1[:, k:k + 1],
                                 start=(k == 0), stop=False)
            nc.tensor.matmul(p2[:], c7_sb[0:1, msl], ones_row[0:1, 0:1],
                             start=False, stop=True)
            t2 = small.tile([128, 1], f32, tag="t2")
            nc.scalar.activation(t2[:], p2[:], Act.Copy)
            nc.vector.scalar_tensor_tensor(
                out=y2[:, mt:mt + 1], in0=t2[:], scalar=SLOPE, in1=t2[:],
                op0=Alu.mult, op1=Alu.max)

        p3 = sm.tile([40, 1], f32, tag="sm")
        for k in range(2):
            nc.tensor.matmul(p3[:], l3_sb[k][:, 0:40], y2[:, k:k + 1],
                             start=(k == 0), stop=False)
        nc.tensor.matmul(p3[:], b8_sb[0:1, 0:40], ones_row[0:1, 0:1],
                         start=False, stop=True)
        y3 = small.tile([40, 1], f32, tag="y3")
        nc.scalar.activation(y3[:], p3[:], Act.Copy)
        nc.sync.dma_start(out_d.ap(), y3[:])

    nc.compile()
    return nc


def _prep_inputs(inputs):
    """Fold eval-mode BN into conv/linear weights; transpose for the device."""
    f = np.float32
    s = lambda g: (g / np.sqrt(f(1.0) + f(EPS))).astype(f)

    def edge(w, g, b, bias=None):
        O, C2 = w.shape
        C = C2 // 2
        sc = s(g)
        Wd = w[:, :C]
        Wc = w[:, C:]
        A = sc[:, None] * Wd
        Bm = sc[:, None] * (Wc - Wd)
        c0 = sc * (bias if bias is not None else 0.0) + b
        return A.T.copy().astype(f), Bm.T.copy().astype(f), c0.reshape(1, -1).astype(f)

    d = {}
    d["AT0"], d["BT0"], d["c00"] = edge(inputs["conv1_w"], inputs["bn1_g"],
                                        inputs["bn1_b"], inputs["conv1_b"])
    d["AT1"], d["BT1"], d["c01"] = edge(inputs["conv2_w"], inputs["bn2_g"], inputs["bn2_b"])
    d["AT2"], d["BT2"], d["c02"] = edge(inputs["conv3_w"], inputs["bn3_g"], inputs["bn3_b"])
    d["AT3"], d["BT3"], d["c03"] = edge(inputs["conv4_w"], inputs["bn4_g"], inputs["bn4_b"])
    d["w5T"] = inputs["conv5_w"].T.copy().astype(f)
    s6 = s(inputs["bn6_g"])
    d["l1T"] = (s6[:, None] * inputs["lin1_w"]).T.copy().astype(f)
    d["b6"] = inputs["bn6_b"].reshape(1, -1).astype(f)
    s7 = s(inputs["bn7_g"])
    d["l2T"] = (s7[:, None] * inputs["lin2_w"]).T.copy().astype(f)
    d["c7"] = (s7 * inputs["lin2_b"] + inputs["bn7_b"]).reshape(1, -1).astype(f)
    d["l3T"] = inputs["lin3_w"].T.copy().astype(f)
    d["b8"] = inputs["lin3_b"].reshape(1, -1).astype(f)
    return d


def _install_ntff_hook():
    """The agent image's antenv lacks axon_hooks; synthesize it and register
    the ctypes NTFF profiling hook from trn_agent_boot (same as trn_boot)."""
    import sys
    import types

    if "antenv.axon_hooks" in sys.modules:
        return
    import antenv

    mod = types.ModuleType("antenv.axon_hooks")
    holder = [None]
    mod.set_axon_ntff_profile_hook = lambda h: holder.__setitem__(0, h)
    mod.get_axon_ntff_profile_hook = lambda: holder[0]
    sys.modules["antenv.axon_hooks"] = mod
    antenv.axon_hooks = mod
    try:
        from trn_agent_boot.trn_boot import _ntff_profile_via_ctypes

        mod.set_axon_ntff_profile_hook(
            _ntff_profile_via_ctypes("/opt/axon/libaxon_pjrt.so"))
    except Exception as e:
        print(f"NTFF hook install failed: {e}")


def kernel(**inputs):
    global LAST_RESULTS
    from concourse.bass_utils import run_bass_kernel_spmd

    if "nc" not in _CACHE:
        _CACHE["nc"] = _build()
    nc = _CACHE["nc"]

    x = np.asarray(inputs["x"], dtype=np.float32)  # (8, 1024, 3)
    common = _prep_inputs({k: np.asarray(v) for k, v in inputs.items()})
    in_maps = [dict(common, xin=np.ascontiguousarray(x[i])) for i in range(NCORES)]

    trace = bool(int(os.environ.get("DGCNN_TRACE", "0")))
    if trace:
        _install_ntff_hook()
    res = run_bass_kernel_spmd(nc, in_maps, core_ids=list(range(NCORES)),
                               trace=trace, trace_cores=[0] if trace else None)
    LAST_RESULTS = res
    out = np.stack([r["out"].reshape(40) for r in res.results]).astype(np.float32)
    return out
